# revision 41
# baseline (speedup 1.0000x reference)
"""Trainium2 Bass kernel for nn_MixingBlock (local-window attention + MLP).

Sharding: 8 cores = (batch 0..3) x (token half 0..1); each core computes
1024 output tokens of one batch element. Attention is local (7x11 window
on the 32x64 token grid), so each core works on a zero-padded 22-image-row
slab (T=1408 tokens) of x and needs no collectives: slab rows for half h
are global rows [16h-3, 16h+19), zero-padded outside [0,32). Queries sit
at slab tokens [192, 1216) and the key window of query tile i is slab
tokens [128i, 128i+512) for BOTH halves, so one SPMD program serves all
cores; window masking uses 5 host-built [128,512] tiles (qt0,qt1,
interior,qt6,qt7 -- edge tiles equal interior on the halves that don't
need them, keeping the program uniform). Per query tile, half the score
PSUMs get an additive -30000 mask folded in by identity matmuls (PE) and
half get a multiplicative {0,1} mask after the exp (split Vector/GpSimd)
to balance engine load.

Device layouts: Q^T/K^T channels-on-partitions ([512f, T], Q pre-scaled),
V token-major with a ones column per head ([T, 33*8]); scores are built
transposed (S^T = K Q^T) so softmax weights feed the AV matmul as
stationary operands without transposes; the ones column yields softmax
denominators inside the same accumulation (all 8 heads of a query tile
accumulate into one [128,264] PSUM; one strided reciprocal + one
broadcast multiply normalize all heads). MLP/LN run token-major
(bn_stats over channels, rstd batched as exp(-0.5*ln(var+eps)) over 4
tiles at a time) with one PE transpose of z1; the attention residual
rides the PSUM->SBUF staging op, and the LN1-gamma residual term rides
the mlp2 matmul via a diagonal gamma matrix. Output is shipped bf16 and
upcast on host.
"""

import contextlib
import sys
import types

import ml_dtypes
import numpy as np

import concourse.bass as bass
import concourse.mybir as mybir
import concourse.tile as tile

# ---------------------------------------------------------------------------
# axon NTFF profile hook (lets run_bass_kernel_spmd(trace=True) work here)
# ---------------------------------------------------------------------------
if "antenv.axon_hooks" not in sys.modules:
    try:
        import antenv  # noqa: F401

        _hookmod = types.ModuleType("antenv.axon_hooks")
        _hookmod._hook = None
        _hookmod.set_axon_ntff_profile_hook = lambda h: setattr(_hookmod, "_hook", h)
        _hookmod.get_axon_ntff_profile_hook = lambda: _hookmod._hook
        sys.modules["antenv.axon_hooks"] = _hookmod
        try:
            from trn_agent_boot.trn_boot import _ntff_profile_via_ctypes

            _hookmod.set_axon_ntff_profile_hook(
                _ntff_profile_via_ctypes("/opt/axon/libaxon_pjrt.so")
            )
        except Exception:
            pass
    except Exception:
        pass

from concourse.bass_utils import run_bass_kernel_spmd  # noqa: E402
from concourse.masks import make_identity  # noqa: E402
from concourse.tile_rust import add_dep_helper  # noqa: E402

F32 = mybir.dt.float32
I32 = mybir.dt.int32
BF16 = mybir.dt.bfloat16
AF = mybir.ActivationFunctionType
ALU = mybir.AluOpType

# Problem constants
H_IMG, W_IMG = 32, 64
N = H_IMG * W_IMG  # 2048
C = 256
NH = 8
HD = 32
HIDDEN = 1024
SCALE = HD**-0.5
B = 4
LN_EPS = 1e-5

# Sharding constants
PAD_ROWS = 3
SLAB_ROWS = 16 + 2 * PAD_ROWS  # 22
T = SLAB_ROWS * W_IMG  # 1408
NQ = 1024
NQT = 8
KW = 512  # key window tokens per query tile
Q0 = PAD_ROWS * W_IMG  # 192: first query token within slab
XCH = (512, 512, 384)  # xT column chunks (token ranges)

MAX_WAITS = 1


def _split_excess_waits(nc):
    """walrus accepts only MAX_WAITS sem-waits per instruction; move excess
    onto same-engine nops inserted immediately before the instruction."""
    for f in nc.m.functions:
        for bb in f.blocks:
            i = 0
            while i < len(bb.instructions):
                ins = bb.instructions[i]
                si = ins.sync_info
                if si is not None and si.on_wait and len(si.on_wait) > MAX_WAITS:
                    waits = list(si.on_wait)
                    extra, keep = waits[:-MAX_WAITS], waits[-MAX_WAITS:]
                    ins.sync_info = mybir.SyncInfo(
                        on_wait=keep, on_update=list(si.on_update)
                    )
                    nops = []
                    for j in range(0, len(extra), MAX_WAITS):
                        nop = nc.engines[ins.engine].nop().ins
                        cur = nc.cur_bb.bb
                        assert cur.instructions[-1] is nop
                        nop.sync_info = mybir.SyncInfo(
                            on_wait=extra[j : j + MAX_WAITS], on_update=[]
                        )
                        cur.instructions.pop()
                        nops.append(nop)
                    bb.instructions[i:i] = nops
                    i += len(nops)
                i += 1


def _build_nc():
    nc = bass.Bass("TRN2", target_bir_lowering=False, num_devices=8)

    d = {}
    d["xT"] = nc.dram_tensor("xT", [C, T], BF16, kind="ExternalInput")
    d["xres"] = nc.dram_tensor("xres", [NQ, C], BF16, kind="ExternalInput")
    d["wqk"] = nc.dram_tensor("wqk", [C, 512], BF16, kind="ExternalInput")
    d["qkb"] = nc.dram_tensor("qkb", [512, 1], F32, kind="ExternalInput")
    d["wv"] = nc.dram_tensor("wv", [C, 264], BF16, kind="ExternalInput")
    d["wp"] = nc.dram_tensor("wp", [C, C], BF16, kind="ExternalInput")
    d["dg1"] = nc.dram_tensor("dg1", [C, C], BF16, kind="ExternalInput")
    d["w1"] = nc.dram_tensor("w1", [C, HIDDEN], BF16, kind="ExternalInput")
    d["b1"] = nc.dram_tensor("b1", [HIDDEN, 1], F32, kind="ExternalInput")
    d["w2"] = nc.dram_tensor("w2", [HIDDEN, C], BF16, kind="ExternalInput")
    d["rows"] = nc.dram_tensor("rows", [1, 3 * C], F32, kind="ExternalInput")
    d["m01"] = nc.dram_tensor("m01", [5, 128, KW], BF16, kind="ExternalInput")
    d["out"] = nc.dram_tensor("out", [NQ, C], BF16, kind="ExternalOutput")

    with tile.TileContext(nc) as tc:
        _emit(nc, tc, d)

    _split_excess_waits(nc)
    return nc


# mask tile index per query tile: qt0, qt1, interior x4, qt6, qt7
M01_OF_QT = [0, 1, 2, 2, 2, 2, 3, 4]


def _emit(nc, tc, d):
    ctx = contextlib.ExitStack()
    with ctx:
        const = ctx.enter_context(tc.tile_pool(name="const", bufs=1))
        big = ctx.enter_context(tc.tile_pool(name="big", bufs=1))
        ps = ctx.enter_context(tc.tile_pool(name="ps", bufs=1, space="PSUM"))
        ps_av = ctx.enter_context(tc.tile_pool(name="ps_av", bufs=1, space="PSUM"))
        ps_t = ps  # transposes/proj ride the 1-buf "mm" ring
        ps_p = ps
        P = {}  # phase-scoped psum pools: "s" (attention) / "h","m2" (mlp)
        work = ctx.enter_context(tc.tile_pool(name="work", bufs=5))
        small = ctx.enter_context(tc.tile_pool(name="small", bufs=6))

        late_dmas = []
        # ---------------- inputs to SBUF (issue order = first use) -------
        # xT row-tiles split into column chunks so phase 1 starts early.
        xT = [
            [
                const.tile([128, w], BF16, tag=f"xT{i}_{c}", name=f"xT{i}_{c}")
                for c, w in enumerate(XCH)
            ]
            for i in range(2)
        ]
        wqk = [const.tile([128, 512], BF16, tag=f"wqk{i}", name=f"wqk{i}") for i in range(2)]
        wv = [const.tile([128, 264], BF16, tag=f"wv{i}", name=f"wv{i}") for i in range(2)]
        wp = [const.tile([128, C], BF16, tag=f"wp{i}", name=f"wp{i}") for i in range(2)]
        dg1 = [const.tile([128, C], BF16, tag=f"dg1{i}", name=f"dg1{i}") for i in range(2)]
        w1 = [const.tile([128, HIDDEN], BF16, tag=f"w1{i}", name=f"w1s{i}") for i in range(2)]
        qkb = [const.tile([128, 1], F32, tag=f"qkb{i}", name=f"qkb{i}") for i in range(4)]
        b1 = [const.tile([128, 1], F32, tag=f"b1{i}", name=f"b1s{i}") for i in range(8)]
        w2 = [const.tile([128, C], BF16, tag=f"w2{i}", name=f"w2s{i}") for i in range(8)]
        xres = [const.tile([128, C], BF16, tag=f"xres{i}", name=f"xres{i}") for i in range(8)]
        m01 = [
            const.tile([128, KW], BF16, tag=f"m01_{i}", name=f"m01_{i}")
            for i in range(5)
        ]
        m30 = [
            const.tile([128, KW], BF16, tag=f"m30_{i}", name=f"m30_{i}")
            for i in range(5)
        ]
        # chunk 0 of xT first (gates the first matmul), wqk/qkb in parallel
        # on the scalar queue.
        off = 0
        for c, w in enumerate(XCH):
            for i in range(2):
                nc.sync.dma_start(out=xT[i][c][:], in_=d["xT"][128 * i : 128 * (i + 1), off : off + w])
            off += w
        for i in range(2):
            nc.scalar.dma_start(out=wqk[i][:], in_=d["wqk"][128 * i : 128 * (i + 1), :])
        for i in range(4):
            nc.scalar.dma_start(out=qkb[i][:], in_=d["qkb"][128 * i : 128 * (i + 1), :])
        for i in range(2):
            nc.gpsimd.dma_start(out=wv[i][:], in_=d["wv"][128 * i : 128 * (i + 1), :])
        for i in range(5):
            late_dmas.append(nc.scalar.dma_start(out=m01[i][:], in_=d["m01"][i]).ins)
        for i in range(2):
            late_dmas.append(nc.scalar.dma_start(out=wp[i][:], in_=d["wp"][128 * i : 128 * (i + 1), :]).ins)
        for i in range(2):
            late_dmas.append(nc.scalar.dma_start(out=dg1[i][:], in_=d["dg1"][128 * i : 128 * (i + 1), :]).ins)
        for i in range(8):
            late_dmas.append(nc.sync.dma_start(
                out=xres[i][:], in_=d["xres"][128 * i : 128 * (i + 1), :]
            ).ins)
        for i in range(2):
            late_dmas.append(nc.sync.dma_start(out=w1[i][:], in_=d["w1"][128 * i : 128 * (i + 1), :]).ins)
        for i in range(8):
            late_dmas.append(nc.sync.dma_start(out=b1[i][:], in_=d["b1"][128 * i : 128 * (i + 1), :]).ins)
        for i in range(8):
            late_dmas.append(nc.sync.dma_start(out=w2[i][:], in_=d["w2"][128 * i : 128 * (i + 1), :]).ins)
        g1b = const.tile([128, C], F32)
        g2b = const.tile([128, C], F32)
        b2lb = const.tile([128, C], F32)
        for t_, lo in ((g1b, 0), (g2b, C), (b2lb, 2 * C)):
            late_dmas.append(
                nc.sync.dma_start(
                    out=t_[:], in_=d["rows"][0:1, lo : lo + C].to_broadcast((128, C))
                ).ins
            )

        eps_t = const.tile([128, 1], F32)
        nc.vector.memset(eps_t[:], LN_EPS)
        ident = const.tile([128, 128], BF16)
        make_identity(nc, ident)
        # additive -30000/0 masks derived on-device from the {0,1} masks
        for i in range(5):
            nc.vector.tensor_scalar(
                out=m30[i][:], in0=m01[i][:], scalar1=-1.0, scalar2=30000.0,
                op0=ALU.add, op1=ALU.mult,
            )

        # ---------------- phase 1: Q^T (scaled) and K^T ----------------
        # K rows (m=2,3) per xT chunk as the chunks arrive; Q rows after.
        qkT = [big.tile([128, T], BF16, tag=f"qkT{m}", name=f"qkT{m}") for m in range(4)]
        first_mm = [None]

        def qk_piece(m, lo, hi, chunk):
            p = P["s"].tile([128, 512], F32, tag="s_ps", name="p_qk")
            w = hi - lo
            coff = sum(XCH[:chunk])
            for cc in range(2):
                mm0 = nc.tensor.matmul(
                    p[:, :w],
                    wqk[cc][:, 128 * m : 128 * (m + 1)],
                    xT[cc][chunk][:, lo - coff : hi - coff],
                    start=(cc == 0),
                    stop=(cc == 1),
                )
                if first_mm[0] is None:
                    first_mm[0] = mm0
                    for dma in late_dmas:
                        add_dep_helper(dma, mm0.ins, sync=True,
                                       reason="defer bulk input DMA")
            nc.vector.tensor_scalar_add(
                out=qkT[m][:, lo:hi], in0=p[:, :w], scalar1=qkb[m][:]
            )

        # ---------------- phase 2: V (token-major, ones columns) ----------
        vt = [big.tile([128, 264], BF16, tag=f"vt{i}", name=f"vt{i}") for i in range(T // 128)]

        def v_tile(i):
            p = P["s"].tile([128, 264], F32, tag="s_ps", name="p_v")
            chunk, coff = i // 4, 128 * (i % 4)
            for cc in range(2):
                nc.tensor.matmul(
                    p[:, :264],
                    xT[cc][chunk][:, coff : coff + 128],
                    wv[cc][:],
                    start=(cc == 0),
                    stop=(cc == 1),
                )
            nc.vector.tensor_copy(vt[i][:], p[:, :264])
            nc.gpsimd.memset(vt[i][:, 32::33], 1.0)

        # ---------------- phase 3: attention ----------------
        attnT = [
            [
                big.tile([128, 128], BF16, tag=f"attnT{j}_{q}", name=f"attnT{j}_{q}")
                for q in range(NQT)
            ]
            for j in range(2)
        ]
        # phase 4 bodies, interleaved into the qt loop (proj of tile t runs
        # while attention works on tile t+1) to keep the PE warm.
        z1 = [big.tile([128, C], BF16, tag=f"z1{i}", name=f"z1_{i}") for i in range(8)]
        z1u = [big.tile([128, C], F32, tag=f"z1u{i}", name=f"z1u_{i}") for i in range(8)]
        mv_a = const.tile([128, 16], F32)
        rstd_a = const.tile([128, 16], F32)

        def proj_ln1(t):
            p_p = ps_p.tile([128, C], F32, tag="mm", name="p_p")
            for cc in range(2):
                nc.tensor.matmul(
                    p_p[:, :C],
                    attnT[cc][t][:],
                    wp[cc][:],
                    start=(cc == 0),
                    stop=(cc == 1),
                )
            nc.vector.scalar_tensor_tensor(
                out=z1u[t][:], in0=p_p[:, :C], scalar=1.0, in1=xres[t][:],
                op0=ALU.mult, op1=ALU.add,
            )
            stats = small.tile([128, 6], F32, tag="stats")
            nc.vector.bn_stats(out=stats[:], in_=z1u[t][:])
            nc.vector.bn_aggr(out=mv_a[:, 2 * t : 2 * t + 2], in_=stats[:])
            if t % 4 == 3:
                g = t // 4
                nc.scalar.activation(
                    out=rstd_a[:, 8 * g : 8 * g + 8], in_=mv_a[:, 8 * g : 8 * g + 8],
                    func=AF.Ln, bias=eps_t[:], scale=1.0,
                )
                nc.scalar.activation(
                    out=rstd_a[:, 8 * g : 8 * g + 8], in_=rstd_a[:, 8 * g : 8 * g + 8],
                    func=AF.Exp, bias=0.0, scale=-0.5,
                )

        z1T = [
            [
                big.tile([128, 512], BF16, tag=f"z1T{j}_{p}", name=f"z1T{j}_{p}")
                for p in range(2)
            ]
            for j in range(2)
        ]

        def z1_fin(t):
            nc.vector.tensor_scalar(
                out=z1[t][:],
                in0=z1u[t][:],
                scalar1=mv_a[:, 2 * t : 2 * t + 1],
                scalar2=rstd_a[:, 2 * t + 1 : 2 * t + 2],
                op0=ALU.subtract,
                op1=ALU.mult,
            )
            for j in range(2):
                p_t = ps_t.tile([128, 128], BF16, tag="mm", name="p_t")
                nc.tensor.transpose(
                    p_t[:, :128], z1[t][:, 128 * j : 128 * (j + 1)], ident[:]
                )
                nc.vector.tensor_copy(
                    z1T[j][t // 4][:, 128 * (t % 4) : 128 * (t % 4 + 1)],
                    p_t[:, :128],
                )

        hT = [
            big.tile([128, 1024], BF16, tag=f"hT{i}", name=f"hT{i}")
            for i in range(8)
        ]

        def mlp1_piece(piece):
            for hc in range(8):
                p_h = P["h"].tile([128, 512], F32, tag="h", name="p_h")
                for cc in range(2):
                    nc.tensor.matmul(
                        p_h[:, :512],
                        w1[cc][:, 128 * hc : 128 * (hc + 1)],
                        z1T[cc][piece][:],
                        start=(cc == 0),
                        stop=(cc == 1),
                    )
                nc.scalar.activation(
                    out=hT[hc][:, 512 * piece : 512 * piece + 512],
                    in_=p_h[:, :512],
                    func=AF.Gelu,
                    bias=b1[hc][:],
                    scale=1.0,
                )

        def attention(qt):
            kw0 = 128 * qt  # key window start token in slab
            attn_q = work.tile([128, NH, 32], BF16, tag="attn_q", name="attn_q")
            p_av8 = ps_av.tile([128, 264], F32, tag="av", name="p_av8")
            for hq in range(2):  # two groups of 4 heads
                heads = [4 * hq + j for j in range(4)]
                p_sA = P["s"].tile([128, 2, KW], F32, tag="s_ps", name="p_sA")
                p_sB = P["s"].tile([128, 2, KW], F32, tag="s_ps", name="p_sB")
                p_of = {heads[0]: (p_sA, 0, False), heads[1]: (p_sA, 1, False),
                        heads[2]: (p_sB, 0, True), heads[3]: (p_sB, 1, True)}
                for c in range(4):
                    for h in (heads[0], heads[2], heads[1], heads[3]):
                        pt_, hi, is_b = p_of[h]
                        ktile, koff = 2 + h // 4, (32 * h) % 128
                        qtile, qoff = h // 4, (32 * h) % 128
                        nc.tensor.matmul(
                            pt_[:, hi, 128 * c : 128 * (c + 1)],
                            qkT[ktile][
                                koff : koff + 32,
                                kw0 + 128 * c : kw0 + 128 * (c + 1),
                            ],
                            qkT[qtile][
                                qoff : qoff + 32,
                                Q0 + 128 * qt : Q0 + 128 * (qt + 1),
                            ],
                            start=(c == 0),
                            stop=(c == 3) and is_b,
                            tile_position=(koff, 0),
                        )
                # additive mask folded into piece A via identity matmuls
                for hi in range(2):
                    nc.tensor.matmul(
                        p_sA[:, hi, :], ident[:], m30[M01_OF_QT[qt]][:],
                        start=False, stop=True,
                    )
                for pi, p_s in enumerate((p_sA, p_sB)):
                    pT = work.tile([128, 2, KW], BF16, tag="pT", name="pT")
                    if pi == 1 and hq == 0:
                        # Schraudolph exp on DVE: bitcast(int32(A*x + B)) ~ e^x
                        # (+-4%); the {0,1} mask multiply absorbs the bitcast.
                        si = work.tile([128, 2, KW], I32, tag="pTi", name="pTi")
                        nc.vector.tensor_scalar(
                            out=si[:], in0=p_s[:],
                            scalar1=12102203.16, scalar2=1064866805.0,
                            op0=ALU.mult, op1=ALU.add,
                        )
                        nc.vector.tensor_mul(
                            pT[:],
                            si[:].bitcast(F32),
                            m01[M01_OF_QT[qt]][:]
                            .rearrange("p (o k) -> p o k", o=1)
                            .to_broadcast((128, 2, KW)),
                        )
                    else:
                        nc.scalar.activation(
                            out=pT[:], in_=p_s[:], func=AF.Exp, bias=0.0, scale=1.0
                        )
                        if pi == 1:
                            nc.gpsimd.tensor_mul(
                                pT[:],
                                pT[:],
                                m01[M01_OF_QT[qt]][:]
                                .rearrange("p (o k) -> p o k", o=1)
                                .to_broadcast((128, 2, KW)),
                            )
                    for hi in range(2):
                        h = heads[2 * pi + hi]
                        for c in range(4):
                            nc.tensor.matmul(
                                p_av8[:, 33 * h : 33 * h + 33],
                                pT[:, hi, 128 * c : 128 * (c + 1)],
                                vt[qt + c][:, 33 * h : 33 * h + 33],
                                start=(c == 0),
                                stop=(c == 3),
                            )
            rec8 = small.tile([128, 8], F32, tag="rec")
            nc.vector.reciprocal(rec8[:], p_av8[:, 32::33])
            nc.vector.tensor_mul(
                attn_q[:],
                p_av8[:].rearrange("p (h x) -> p h x", h=8)[:, :, 0:32],
                rec8[:].rearrange("p (h o) -> p h o", o=1).to_broadcast((128, 8, 32)),
            )
            for j in range(2):
                p_t2 = ps_t.tile([128, 128], BF16, tag="mm", name="p_t2")
                nc.tensor.transpose(
                    p_t2[:, :128],
                    attn_q[:, 4 * j : 4 * (j + 1), :].rearrange("p a b -> p (a b)"),
                    ident[:],
                )
                nc.vector.tensor_copy(attnT[j][qt][:], p_t2[:, :128])

        # ------------- emission schedule -------------
        ps_s_cm = tc.tile_pool(name="ps_s", bufs=3, space="PSUM")
        P["s"] = ps_s_cm.__enter__()
        qk_piece(2, 0, 512, 0)
        qk_piece(3, 0, 512, 0)
        qk_piece(2, 512, 1024, 1)
        qk_piece(3, 512, 1024, 1)
        qk_piece(0, Q0, 512, 0)
        qk_piece(1, Q0, 512, 0)
        qk_piece(0, 512, 1024, 1)
        qk_piece(1, 512, 1024, 1)
        qk_piece(2, 1024, T, 2)
        qk_piece(3, 1024, T, 2)
        qk_piece(0, 1024, Q0 + NQ, 2)
        qk_piece(1, 1024, Q0 + NQ, 2)
        for i in range(11):
            v_tile(i)
        attention(0)
        attention(1)
        proj_ln1(0)
        attention(2)
        proj_ln1(1)
        attention(3)
        proj_ln1(2)
        attention(4)
        proj_ln1(3)
        attention(5)
        proj_ln1(4)
        z1_fin(0)
        z1_fin(1)
        attention(6)
        proj_ln1(5)
        z1_fin(2)
        z1_fin(3)
        attention(7)
        proj_ln1(6)
        ps_s_cm.__exit__(None, None, None)
        ps_h_cm = tc.tile_pool(name="ps_h", bufs=3, space="PSUM")
        P["h"] = ps_h_cm.__enter__()
        ps_m2_cm = tc.tile_pool(name="ps_m2", bufs=3, space="PSUM")
        P["m2"] = ps_m2_cm.__enter__()
        # ---------------- phase 6: mlp2 + resid2 + LN2 + out -------------
        # p_m accumulates mlp2 AND the gamma1*z1 residual (diag matmul);
        # bv2 rides the PSUM->SBUF staging op.
        mv_b = const.tile([128, 16], F32)
        rstd_b = const.tile([128, 16], F32)
        r2 = [big.tile([128, C], F32, tag=f"r2_{t}", name=f"r2_{t}") for t in range(8)]

        def mlp2_tile(t):
            p_m = P["m2"].tile([128, C], F32, tag="m2", name="p_m")
            tok0 = 512 * (t // 4) + 128 * (t % 4)
            for hc in range(8):
                nc.tensor.matmul(
                    p_m[:, :C],
                    hT[hc][:, tok0 : tok0 + 128],
                    w2[hc][:],
                    start=(hc == 0),
                    stop=False,
                )
            for cc in range(2):
                nc.tensor.matmul(
                    p_m[:, :C],
                    z1T[cc][t // 4][:, 128 * (t % 4) : 128 * (t % 4 + 1)],
                    dg1[cc][:],
                    start=False,
                    stop=(cc == 1),
                )
            nc.vector.scalar_tensor_tensor(
                out=r2[t][:], in0=p_m[:, :C], scalar=1.0, in1=g1b[:],
                op0=ALU.mult, op1=ALU.add,
            )
            stats = small.tile([128, 6], F32, tag="stats2")
            nc.vector.bn_stats(out=stats[:], in_=r2[t][:])
            nc.vector.bn_aggr(out=mv_b[:, 2 * t : 2 * t + 2], in_=stats[:])
            if t % 2 == 1:
                g = t // 2
                nc.scalar.activation(
                    out=rstd_b[:, 4 * g : 4 * g + 4], in_=mv_b[:, 4 * g : 4 * g + 4],
                    func=AF.Ln, bias=eps_t[:], scale=1.0,
                )
                nc.scalar.activation(
                    out=rstd_b[:, 4 * g : 4 * g + 4], in_=rstd_b[:, 4 * g : 4 * g + 4],
                    func=AF.Exp, bias=0.0, scale=-0.5,
                )
                for u in range(2 * g, 2 * g + 2):
                    z2 = work.tile([128, C], F32, tag="z2")
                    nc.vector.tensor_scalar(
                        out=z2[:],
                        in0=r2[u][:],
                        scalar1=mv_b[:, 2 * u : 2 * u + 1],
                        scalar2=rstd_b[:, 2 * u + 1 : 2 * u + 2],
                        op0=ALU.subtract,
                        op1=ALU.mult,
                    )
                    o1 = work.tile([128, C], F32, tag="o1")
                    nc.gpsimd.tensor_mul(o1[:], z2[:], g2b[:])
                    o = work.tile([128, C], BF16, tag="o")
                    nc.gpsimd.tensor_add(o[:], o1[:], b2lb[:])
                    nc.sync.dma_start(out=d["out"][128 * u : 128 * (u + 1), :], in_=o[:])

        proj_ln1(7)
        mlp1_piece(0)
        for t in range(4, 8):
            z1_fin(t)
        for t in range(4):
            mlp2_tile(t)
        mlp1_piece(1)
        for t in range(4, 8):
            mlp2_tile(t)
        ps_m2_cm.__exit__(None, None, None)
        ps_h_cm.__exit__(None, None, None)


_NC_CACHE = None
_LAST_RESULT = None


def _get_nc():
    global _NC_CACHE
    if _NC_CACHE is None:
        _NC_CACHE = _build_nc()
    return _NC_CACHE


def _to_bf16(a):
    return np.ascontiguousarray(np.asarray(a, dtype=np.float32)).astype(
        ml_dtypes.bfloat16
    )


def _host_inputs(core, x, mask, qkv_w, qkv_b, proj_w, proj_b, ln1_g, ln1_b, w1,
                 b1, w2, b2, ln2_g, ln2_b):
    b = core // 2
    half = core % 2
    row0 = 16 * half - PAD_ROWS  # slab start image row (may be negative)
    S0 = row0 * W_IMG  # slab start token
    Q0g = 1024 * half  # first query token (global)

    xb = np.asarray(x[b], dtype=np.float32)  # [N, C]
    slab = np.zeros((T, C), np.float32)
    g_lo, g_hi = max(0, S0), min(N, S0 + T)
    slab[g_lo - S0 : g_hi - S0] = xb[g_lo:g_hi]

    wqk = np.concatenate([qkv_w[:C] * SCALE, qkv_w[C : 2 * C]], axis=0)  # [512,C]
    qkb = np.concatenate([qkv_b[:C] * SCALE, qkv_b[C : 2 * C]])[:, None]
    wv = qkv_w[2 * C :]  # [256, 256]
    vb = qkv_b[2 * C :]
    assert np.abs(vb).max() == 0.0, "nonzero v bias not folded"
    wv_pad = np.zeros((C, 264), np.float32)
    for h in range(NH):
        wv_pad[:, 33 * h : 33 * h + 32] = wv[32 * h : 32 * h + 32].T

    w1f = w1 * ln1_g[None, :]  # fold ln1 gamma
    b1f = (b1 + w1 @ ln1_b)[:, None]  # fold ln1 beta (mlp path)
    bvec2 = b2 + ln1_b  # resid2 constant (residual path)

    xres = xb[Q0g : Q0g + NQ] + proj_b[None, :]

    # {0,1} masks: 5 tiles [128, 512] (qt0, qt1, interior, qt6, qt7);
    # v[p, 128c+q] = valid(key (c,p), query q)
    v5 = np.zeros((5, 128, KW), np.float32)

    def _vt_of(i):
        qg = Q0g + 128 * i
        valid = np.zeros((128, KW), np.float32)  # [q, k-in-window]
        for r in range(8):
            gr = row0 + 2 * i + r  # global image row of window row r
            if 0 <= gr < H_IMG:
                valid[:, 64 * r : 64 * (r + 1)] = (
                    mask[qg : qg + 128, 64 * gr : 64 * (gr + 1)] == 0
                )
        # coverage check: every allowed key lies inside the window
        full = mask[qg : qg + 128] == 0
        assert int(full.sum()) == int(valid.sum()), (core, i, "window coverage")
        return valid.T.reshape(4, 128, 128).transpose(1, 0, 2).reshape(128, KW)

    for sl, i in enumerate((0, 1, 2, 6, 7)):
        v5[sl] = _vt_of(i)
    for i in (3, 4, 5):  # interior coherence
        assert (v5[2] == _vt_of(i)).all(), (core, i, "interior mask mismatch")

    rows = np.concatenate([bvec2, ln2_g, ln2_b])[None, :]

    return {
        "xT": _to_bf16(slab.T),
        "xres": _to_bf16(xres),
        "wqk": _to_bf16(wqk.T),
        "qkb": np.ascontiguousarray(qkb, dtype=np.float32),
        "wv": _to_bf16(wv_pad),
        "wp": _to_bf16(proj_w.T),
        "dg1": _to_bf16(np.diag(ln1_g)),
        "w1": _to_bf16(w1f.T),
        "b1": np.ascontiguousarray(b1f, dtype=np.float32),
        "w2": _to_bf16(w2.T),
        "rows": np.ascontiguousarray(rows, dtype=np.float32),
        "m01": _to_bf16(v5),
    }


def kernel(**inputs):
    args = {k: np.asarray(v) for k, v in inputs.items()}
    in_maps = [
        _host_inputs(
            core,
            args["x"],
            np.asarray(args["mask"], dtype=np.float32),
            args["qkv_w"],
            args["qkv_b"],
            args["proj_w"],
            args["proj_b"],
            args["ln1_g"],
            args["ln1_b"],
            args["w1"],
            args["b1"],
            args["w2"],
            args["b2"],
            args["ln2_g"],
            args["ln2_b"],
        )
        for core in range(8)
    ]
    nc = _get_nc()
    res = run_bass_kernel_spmd(nc, in_maps, core_ids=list(range(8)))
    global _LAST_RESULT
    _LAST_RESULT = res
    out = np.zeros((B, N, C), np.float32)
    for core in range(8):
        b, half = core // 2, core % 2
        out[b, 1024 * half : 1024 * (half + 1)] = np.asarray(
            res.results[core]["out"], dtype=np.float32
        )
    return out


# revision 42
# speedup vs baseline: 1.0061x; 1.0061x over previous
"""Trainium2 Bass kernel for nn_MixingBlock (local-window attention + MLP).

Sharding: 8 cores = (batch 0..3) x (token half 0..1); each core computes
1024 output tokens of one batch element. Attention is local (7x11 window
on the 32x64 token grid), so each core works on a zero-padded 22-image-row
slab (T=1408 tokens) of x and needs no collectives: slab rows for half h
are global rows [16h-3, 16h+19), zero-padded outside [0,32). Queries sit
at slab tokens [192, 1216) and the key window of query tile i is slab
tokens [128i, 128i+512) for BOTH halves, so one SPMD program serves all
cores; window masking uses 5 host-built [128,512] tiles (qt0,qt1,
interior,qt6,qt7 -- edge tiles equal interior on the halves that don't
need them, keeping the program uniform). Per query tile, half the score
PSUMs get an additive -30000 mask folded in by identity matmuls (PE) and
half get a multiplicative {0,1} mask after the exp (split Vector/GpSimd)
to balance engine load.

Device layouts: Q^T/K^T channels-on-partitions ([512f, T], Q pre-scaled),
V token-major with a ones column per head ([T, 33*8]); scores are built
transposed (S^T = K Q^T) so softmax weights feed the AV matmul as
stationary operands without transposes; the ones column yields softmax
denominators inside the same accumulation (all 8 heads of a query tile
accumulate into one [128,264] PSUM; one strided reciprocal + one
broadcast multiply normalize all heads). MLP/LN run token-major
(bn_stats over channels, rstd batched as exp(-0.5*ln(var+eps)) over 4
tiles at a time) with one PE transpose of z1; the attention residual
rides the PSUM->SBUF staging op, and the LN1-gamma residual term rides
the mlp2 matmul via a diagonal gamma matrix. Output is shipped bf16 and
upcast on host.
"""

import contextlib
import sys
import types

import ml_dtypes
import numpy as np

import concourse.bass as bass
import concourse.mybir as mybir
import concourse.tile as tile

# ---------------------------------------------------------------------------
# axon NTFF profile hook (lets run_bass_kernel_spmd(trace=True) work here)
# ---------------------------------------------------------------------------
if "antenv.axon_hooks" not in sys.modules:
    try:
        import antenv  # noqa: F401

        _hookmod = types.ModuleType("antenv.axon_hooks")
        _hookmod._hook = None
        _hookmod.set_axon_ntff_profile_hook = lambda h: setattr(_hookmod, "_hook", h)
        _hookmod.get_axon_ntff_profile_hook = lambda: _hookmod._hook
        sys.modules["antenv.axon_hooks"] = _hookmod
        try:
            from trn_agent_boot.trn_boot import _ntff_profile_via_ctypes

            _hookmod.set_axon_ntff_profile_hook(
                _ntff_profile_via_ctypes("/opt/axon/libaxon_pjrt.so")
            )
        except Exception:
            pass
    except Exception:
        pass

from concourse.bass_utils import run_bass_kernel_spmd  # noqa: E402
from concourse.masks import make_identity  # noqa: E402
from concourse.tile_rust import add_dep_helper  # noqa: E402

F32 = mybir.dt.float32
BF16 = mybir.dt.bfloat16
AF = mybir.ActivationFunctionType
ALU = mybir.AluOpType

# Problem constants
H_IMG, W_IMG = 32, 64
N = H_IMG * W_IMG  # 2048
C = 256
NH = 8
HD = 32
HIDDEN = 1024
SCALE = HD**-0.5
B = 4
LN_EPS = 1e-5

# Sharding constants
PAD_ROWS = 3
SLAB_ROWS = 16 + 2 * PAD_ROWS  # 22
T = SLAB_ROWS * W_IMG  # 1408
NQ = 1024
NQT = 8
KW = 512  # key window tokens per query tile
Q0 = PAD_ROWS * W_IMG  # 192: first query token within slab
XCH = (512, 512, 384)  # xT column chunks (token ranges)

MAX_WAITS = 1


def _split_excess_waits(nc):
    """walrus accepts only MAX_WAITS sem-waits per instruction; move excess
    onto same-engine nops inserted immediately before the instruction."""
    for f in nc.m.functions:
        for bb in f.blocks:
            i = 0
            while i < len(bb.instructions):
                ins = bb.instructions[i]
                si = ins.sync_info
                if si is not None and si.on_wait and len(si.on_wait) > MAX_WAITS:
                    waits = list(si.on_wait)
                    extra, keep = waits[:-MAX_WAITS], waits[-MAX_WAITS:]
                    ins.sync_info = mybir.SyncInfo(
                        on_wait=keep, on_update=list(si.on_update)
                    )
                    nops = []
                    for j in range(0, len(extra), MAX_WAITS):
                        nop = nc.engines[ins.engine].nop().ins
                        cur = nc.cur_bb.bb
                        assert cur.instructions[-1] is nop
                        nop.sync_info = mybir.SyncInfo(
                            on_wait=extra[j : j + MAX_WAITS], on_update=[]
                        )
                        cur.instructions.pop()
                        nops.append(nop)
                    bb.instructions[i:i] = nops
                    i += len(nops)
                i += 1


def _build_nc():
    nc = bass.Bass("TRN2", target_bir_lowering=False, num_devices=8)

    d = {}
    d["xT"] = nc.dram_tensor("xT", [C, T], BF16, kind="ExternalInput")
    d["xres"] = nc.dram_tensor("xres", [NQ, C], BF16, kind="ExternalInput")
    d["wqk"] = nc.dram_tensor("wqk", [C, 512], BF16, kind="ExternalInput")
    d["qkb"] = nc.dram_tensor("qkb", [512, 1], F32, kind="ExternalInput")
    d["wv"] = nc.dram_tensor("wv", [C, 264], BF16, kind="ExternalInput")
    d["wp"] = nc.dram_tensor("wp", [C, C], BF16, kind="ExternalInput")
    d["dg1"] = nc.dram_tensor("dg1", [C, C], BF16, kind="ExternalInput")
    d["w1"] = nc.dram_tensor("w1", [C, HIDDEN], BF16, kind="ExternalInput")
    d["b1"] = nc.dram_tensor("b1", [HIDDEN, 1], F32, kind="ExternalInput")
    d["w2"] = nc.dram_tensor("w2", [HIDDEN, C], BF16, kind="ExternalInput")
    d["rows"] = nc.dram_tensor("rows", [1, 3 * C], F32, kind="ExternalInput")
    d["m01"] = nc.dram_tensor("m01", [5, 128, KW], BF16, kind="ExternalInput")
    d["out"] = nc.dram_tensor("out", [NQ, C], BF16, kind="ExternalOutput")

    with tile.TileContext(nc) as tc:
        _emit(nc, tc, d)

    _split_excess_waits(nc)
    return nc


# mask tile index per query tile: qt0, qt1, interior x4, qt6, qt7
M01_OF_QT = [0, 1, 2, 2, 2, 2, 3, 4]


def _emit(nc, tc, d):
    ctx = contextlib.ExitStack()
    with ctx:
        const = ctx.enter_context(tc.tile_pool(name="const", bufs=1))
        big = ctx.enter_context(tc.tile_pool(name="big", bufs=1))
        ps = ctx.enter_context(tc.tile_pool(name="ps", bufs=1, space="PSUM"))
        ps_av = ctx.enter_context(tc.tile_pool(name="ps_av", bufs=1, space="PSUM"))
        ps_t = ps  # transposes/proj ride the 1-buf "mm" ring
        ps_p = ps
        P = {}  # phase-scoped psum pools: "s" (attention) / "h","m2" (mlp)
        work = ctx.enter_context(tc.tile_pool(name="work", bufs=5))
        small = ctx.enter_context(tc.tile_pool(name="small", bufs=6))

        late_dmas = []
        # ---------------- inputs to SBUF (issue order = first use) -------
        # xT row-tiles split into column chunks so phase 1 starts early.
        xT = [
            [
                const.tile([128, w], BF16, tag=f"xT{i}_{c}", name=f"xT{i}_{c}")
                for c, w in enumerate(XCH)
            ]
            for i in range(2)
        ]
        wqk = [const.tile([128, 512], BF16, tag=f"wqk{i}", name=f"wqk{i}") for i in range(2)]
        wv = [const.tile([128, 264], BF16, tag=f"wv{i}", name=f"wv{i}") for i in range(2)]
        wp = [const.tile([128, C], BF16, tag=f"wp{i}", name=f"wp{i}") for i in range(2)]
        dg1 = [const.tile([128, C], BF16, tag=f"dg1{i}", name=f"dg1{i}") for i in range(2)]
        w1 = [const.tile([128, HIDDEN], BF16, tag=f"w1{i}", name=f"w1s{i}") for i in range(2)]
        qkb = [const.tile([128, 1], F32, tag=f"qkb{i}", name=f"qkb{i}") for i in range(4)]
        b1 = [const.tile([128, 1], F32, tag=f"b1{i}", name=f"b1s{i}") for i in range(8)]
        w2 = [const.tile([128, C], BF16, tag=f"w2{i}", name=f"w2s{i}") for i in range(8)]
        xres = [const.tile([128, C], BF16, tag=f"xres{i}", name=f"xres{i}") for i in range(8)]
        m01 = [
            const.tile([128, KW], BF16, tag=f"m01_{i}", name=f"m01_{i}")
            for i in range(5)
        ]
        m30 = [
            const.tile([128, KW], BF16, tag=f"m30_{i}", name=f"m30_{i}")
            for i in range(5)
        ]
        # chunk 0 of xT first (gates the first matmul), wqk/qkb in parallel
        # on the scalar queue.
        off = 0
        for c, w in enumerate(XCH):
            for i in range(2):
                nc.sync.dma_start(out=xT[i][c][:], in_=d["xT"][128 * i : 128 * (i + 1), off : off + w])
            off += w
        for i in range(2):
            nc.scalar.dma_start(out=wqk[i][:], in_=d["wqk"][128 * i : 128 * (i + 1), :])
        for i in range(4):
            nc.scalar.dma_start(out=qkb[i][:], in_=d["qkb"][128 * i : 128 * (i + 1), :])
        for i in range(2):
            nc.gpsimd.dma_start(out=wv[i][:], in_=d["wv"][128 * i : 128 * (i + 1), :])
        for i in range(5):
            late_dmas.append(nc.scalar.dma_start(out=m01[i][:], in_=d["m01"][i]).ins)
        for i in range(2):
            late_dmas.append(nc.scalar.dma_start(out=wp[i][:], in_=d["wp"][128 * i : 128 * (i + 1), :]).ins)
        for i in range(2):
            late_dmas.append(nc.scalar.dma_start(out=dg1[i][:], in_=d["dg1"][128 * i : 128 * (i + 1), :]).ins)
        for i in range(8):
            late_dmas.append(nc.sync.dma_start(
                out=xres[i][:], in_=d["xres"][128 * i : 128 * (i + 1), :]
            ).ins)
        for i in range(2):
            late_dmas.append(nc.sync.dma_start(out=w1[i][:], in_=d["w1"][128 * i : 128 * (i + 1), :]).ins)
        for i in range(8):
            late_dmas.append(nc.sync.dma_start(out=b1[i][:], in_=d["b1"][128 * i : 128 * (i + 1), :]).ins)
        for i in range(8):
            late_dmas.append(nc.sync.dma_start(out=w2[i][:], in_=d["w2"][128 * i : 128 * (i + 1), :]).ins)
        g1b = const.tile([128, C], F32)
        g2b = const.tile([128, C], F32)
        b2lb = const.tile([128, C], F32)
        for t_, lo in ((g1b, 0), (g2b, C), (b2lb, 2 * C)):
            late_dmas.append(
                nc.sync.dma_start(
                    out=t_[:], in_=d["rows"][0:1, lo : lo + C].to_broadcast((128, C))
                ).ins
            )

        eps_t = const.tile([128, 1], F32)
        nc.vector.memset(eps_t[:], LN_EPS)
        ident = const.tile([128, 128], BF16)
        make_identity(nc, ident)
        # additive -30000/0 masks derived on-device from the {0,1} masks
        for i in range(5):
            nc.vector.tensor_scalar(
                out=m30[i][:], in0=m01[i][:], scalar1=-1.0, scalar2=30000.0,
                op0=ALU.add, op1=ALU.mult,
            )

        # ---------------- phase 1: Q^T (scaled) and K^T ----------------
        # K rows (m=2,3) per xT chunk as the chunks arrive; Q rows after.
        qkT = [big.tile([128, T], BF16, tag=f"qkT{m}", name=f"qkT{m}") for m in range(4)]
        first_mm = [None]

        def qk_piece(m, lo, hi, chunk):
            p = P["s"].tile([128, 512], F32, tag="s_ps", name="p_qk")
            w = hi - lo
            coff = sum(XCH[:chunk])
            for cc in range(2):
                mm0 = nc.tensor.matmul(
                    p[:, :w],
                    wqk[cc][:, 128 * m : 128 * (m + 1)],
                    xT[cc][chunk][:, lo - coff : hi - coff],
                    start=(cc == 0),
                    stop=(cc == 1),
                )
                if first_mm[0] is None:
                    first_mm[0] = mm0
                    for dma in late_dmas:
                        add_dep_helper(dma, mm0.ins, sync=True,
                                       reason="defer bulk input DMA")
            nc.vector.tensor_scalar_add(
                out=qkT[m][:, lo:hi], in0=p[:, :w], scalar1=qkb[m][:]
            )

        # ---------------- phase 2: V (token-major, ones columns) ----------
        vt = [big.tile([128, 264], BF16, tag=f"vt{i}", name=f"vt{i}") for i in range(T // 128)]

        def v_tile(i):
            p = P["s"].tile([128, 264], F32, tag="s_ps", name="p_v")
            chunk, coff = i // 4, 128 * (i % 4)
            for cc in range(2):
                nc.tensor.matmul(
                    p[:, :264],
                    xT[cc][chunk][:, coff : coff + 128],
                    wv[cc][:],
                    start=(cc == 0),
                    stop=(cc == 1),
                )
            nc.vector.tensor_copy(vt[i][:], p[:, :264])
            nc.gpsimd.memset(vt[i][:, 32::33], 1.0)

        # ---------------- phase 3: attention ----------------
        attnT = [
            [
                big.tile([128, 128], BF16, tag=f"attnT{j}_{q}", name=f"attnT{j}_{q}")
                for q in range(NQT)
            ]
            for j in range(2)
        ]
        # phase 4 bodies, interleaved into the qt loop (proj of tile t runs
        # while attention works on tile t+1) to keep the PE warm.
        z1 = [big.tile([128, C], BF16, tag=f"z1{i}", name=f"z1_{i}") for i in range(8)]
        z1u = [big.tile([128, C], F32, tag=f"z1u{i}", name=f"z1u_{i}") for i in range(8)]
        mv_a = const.tile([128, 16], F32)
        rstd_a = const.tile([128, 16], F32)

        def proj_ln1(t):
            p_p = ps_p.tile([128, C], F32, tag="mm", name="p_p")
            for cc in range(2):
                nc.tensor.matmul(
                    p_p[:, :C],
                    attnT[cc][t][:],
                    wp[cc][:],
                    start=(cc == 0),
                    stop=(cc == 1),
                )
            nc.vector.scalar_tensor_tensor(
                out=z1u[t][:], in0=p_p[:, :C], scalar=1.0, in1=xres[t][:],
                op0=ALU.mult, op1=ALU.add,
            )
            stats = small.tile([128, 6], F32, tag="stats")
            nc.vector.bn_stats(out=stats[:], in_=z1u[t][:])
            nc.vector.bn_aggr(out=mv_a[:, 2 * t : 2 * t + 2], in_=stats[:])
            if t % 4 == 3:
                g = t // 4
                nc.scalar.activation(
                    out=rstd_a[:, 8 * g : 8 * g + 8], in_=mv_a[:, 8 * g : 8 * g + 8],
                    func=AF.Ln, bias=eps_t[:], scale=1.0,
                )
                nc.scalar.activation(
                    out=rstd_a[:, 8 * g : 8 * g + 8], in_=rstd_a[:, 8 * g : 8 * g + 8],
                    func=AF.Exp, bias=0.0, scale=-0.5,
                )

        z1T = [
            [
                big.tile([128, 512], BF16, tag=f"z1T{j}_{p}", name=f"z1T{j}_{p}")
                for p in range(2)
            ]
            for j in range(2)
        ]

        def z1_fin(t):
            nc.vector.tensor_scalar(
                out=z1[t][:],
                in0=z1u[t][:],
                scalar1=mv_a[:, 2 * t : 2 * t + 1],
                scalar2=rstd_a[:, 2 * t + 1 : 2 * t + 2],
                op0=ALU.subtract,
                op1=ALU.mult,
            )
            for j in range(2):
                p_t = ps_t.tile([128, 128], BF16, tag="mm", name="p_t")
                nc.tensor.transpose(
                    p_t[:, :128], z1[t][:, 128 * j : 128 * (j + 1)], ident[:]
                )
                nc.vector.tensor_copy(
                    z1T[j][t // 4][:, 128 * (t % 4) : 128 * (t % 4 + 1)],
                    p_t[:, :128],
                )

        hT = [
            big.tile([128, 1024], BF16, tag=f"hT{i}", name=f"hT{i}")
            for i in range(8)
        ]

        def mlp1_piece(piece):
            for hc in range(8):
                p_h = P["h"].tile([128, 512], F32, tag="h", name="p_h")
                for cc in range(2):
                    nc.tensor.matmul(
                        p_h[:, :512],
                        w1[cc][:, 128 * hc : 128 * (hc + 1)],
                        z1T[cc][piece][:],
                        start=(cc == 0),
                        stop=(cc == 1),
                    )
                nc.scalar.activation(
                    out=hT[hc][:, 512 * piece : 512 * piece + 512],
                    in_=p_h[:, :512],
                    func=AF.Gelu,
                    bias=b1[hc][:],
                    scale=1.0,
                )

        def attention(qt):
            kw0 = 128 * qt  # key window start token in slab
            attn_q = work.tile([128, NH, 32], BF16, tag="attn_q", name="attn_q")
            p_av8 = ps_av.tile([128, 264], F32, tag="av", name="p_av8")
            for hq in range(2):  # two groups of 4 heads
                heads = [4 * hq + j for j in range(4)]
                p_sA = P["s"].tile([128, 2, KW], F32, tag="s_ps", name="p_sA")
                p_sB = P["s"].tile([128, 2, KW], F32, tag="s_ps", name="p_sB")
                p_of = {heads[0]: (p_sA, 0, False), heads[1]: (p_sA, 1, False),
                        heads[2]: (p_sB, 0, True), heads[3]: (p_sB, 1, True)}
                # additive mask pre-fills piece A (off the scores->exp path)
                for hi in range(2):
                    nc.tensor.matmul(
                        p_sA[:, hi, :], ident[:], m30[M01_OF_QT[qt]][:],
                        start=True, stop=False,
                    )
                for c in range(4):
                    for h in (heads[0], heads[2], heads[1], heads[3]):
                        pt_, hi, is_b = p_of[h]
                        ktile, koff = 2 + h // 4, (32 * h) % 128
                        qtile, qoff = h // 4, (32 * h) % 128
                        nc.tensor.matmul(
                            pt_[:, hi, 128 * c : 128 * (c + 1)],
                            qkT[ktile][
                                koff : koff + 32,
                                kw0 + 128 * c : kw0 + 128 * (c + 1),
                            ],
                            qkT[qtile][
                                qoff : qoff + 32,
                                Q0 + 128 * qt : Q0 + 128 * (qt + 1),
                            ],
                            start=(c == 0) and is_b,
                            stop=(c == 3),
                            tile_position=(koff, 0),
                        )
                for pi, p_s in enumerate((p_sA, p_sB)):
                    pT = work.tile([128, 2, KW], BF16, tag="pT", name="pT")
                    nc.scalar.activation(
                        out=pT[:], in_=p_s[:], func=AF.Exp, bias=0.0, scale=1.0
                    )
                    if pi == 1:  # piece B: multiplicative {0,1} mask
                        meng = nc.vector if hq == 0 else nc.gpsimd
                        meng.tensor_mul(
                            pT[:],
                            pT[:],
                            m01[M01_OF_QT[qt]][:]
                            .rearrange("p (o k) -> p o k", o=1)
                            .to_broadcast((128, 2, KW)),
                        )
                    for hi in range(2):
                        h = heads[2 * pi + hi]
                        for c in range(4):
                            nc.tensor.matmul(
                                p_av8[:, 33 * h : 33 * h + 33],
                                pT[:, hi, 128 * c : 128 * (c + 1)],
                                vt[qt + c][:, 33 * h : 33 * h + 33],
                                start=(c == 0),
                                stop=(c == 3),
                            )
            rec8 = small.tile([128, 8], F32, tag="rec")
            nc.vector.reciprocal(rec8[:], p_av8[:, 32::33])
            nc.vector.tensor_mul(
                attn_q[:],
                p_av8[:].rearrange("p (h x) -> p h x", h=8)[:, :, 0:32],
                rec8[:].rearrange("p (h o) -> p h o", o=1).to_broadcast((128, 8, 32)),
            )
            for j in range(2):
                p_t2 = ps_t.tile([128, 128], BF16, tag="mm", name="p_t2")
                nc.tensor.transpose(
                    p_t2[:, :128],
                    attn_q[:, 4 * j : 4 * (j + 1), :].rearrange("p a b -> p (a b)"),
                    ident[:],
                )
                nc.vector.tensor_copy(attnT[j][qt][:], p_t2[:, :128])

        # ------------- emission schedule -------------
        ps_s_cm = tc.tile_pool(name="ps_s", bufs=3, space="PSUM")
        P["s"] = ps_s_cm.__enter__()
        qk_piece(2, 0, 512, 0)
        qk_piece(3, 0, 512, 0)
        qk_piece(2, 512, 1024, 1)
        qk_piece(3, 512, 1024, 1)
        qk_piece(0, Q0, 512, 0)
        qk_piece(1, Q0, 512, 0)
        qk_piece(0, 512, 1024, 1)
        qk_piece(1, 512, 1024, 1)
        qk_piece(2, 1024, T, 2)
        qk_piece(3, 1024, T, 2)
        qk_piece(0, 1024, Q0 + NQ, 2)
        qk_piece(1, 1024, Q0 + NQ, 2)
        for i in range(11):
            v_tile(i)
        attention(0)
        attention(1)
        proj_ln1(0)
        attention(2)
        proj_ln1(1)
        attention(3)
        proj_ln1(2)
        attention(4)
        proj_ln1(3)
        attention(5)
        proj_ln1(4)
        z1_fin(0)
        z1_fin(1)
        attention(6)
        proj_ln1(5)
        z1_fin(2)
        z1_fin(3)
        attention(7)
        proj_ln1(6)
        ps_s_cm.__exit__(None, None, None)
        ps_h_cm = tc.tile_pool(name="ps_h", bufs=3, space="PSUM")
        P["h"] = ps_h_cm.__enter__()
        ps_m2_cm = tc.tile_pool(name="ps_m2", bufs=3, space="PSUM")
        P["m2"] = ps_m2_cm.__enter__()
        # ---------------- phase 6: mlp2 + resid2 + LN2 + out -------------
        # p_m accumulates mlp2 AND the gamma1*z1 residual (diag matmul);
        # bv2 rides the PSUM->SBUF staging op.
        mv_b = const.tile([128, 16], F32)
        rstd_b = const.tile([128, 16], F32)
        r2 = [big.tile([128, C], F32, tag=f"r2_{t}", name=f"r2_{t}") for t in range(8)]

        def mlp2_tile(t):
            p_m = P["m2"].tile([128, C], F32, tag="m2", name="p_m")
            tok0 = 512 * (t // 4) + 128 * (t % 4)
            for hc in range(8):
                nc.tensor.matmul(
                    p_m[:, :C],
                    hT[hc][:, tok0 : tok0 + 128],
                    w2[hc][:],
                    start=(hc == 0),
                    stop=False,
                )
            for cc in range(2):
                nc.tensor.matmul(
                    p_m[:, :C],
                    z1T[cc][t // 4][:, 128 * (t % 4) : 128 * (t % 4 + 1)],
                    dg1[cc][:],
                    start=False,
                    stop=(cc == 1),
                )
            nc.vector.scalar_tensor_tensor(
                out=r2[t][:], in0=p_m[:, :C], scalar=1.0, in1=g1b[:],
                op0=ALU.mult, op1=ALU.add,
            )
            stats = small.tile([128, 6], F32, tag="stats2")
            nc.vector.bn_stats(out=stats[:], in_=r2[t][:])
            nc.vector.bn_aggr(out=mv_b[:, 2 * t : 2 * t + 2], in_=stats[:])
            if t % 2 == 1:
                g = t // 2
                nc.scalar.activation(
                    out=rstd_b[:, 4 * g : 4 * g + 4], in_=mv_b[:, 4 * g : 4 * g + 4],
                    func=AF.Ln, bias=eps_t[:], scale=1.0,
                )
                nc.scalar.activation(
                    out=rstd_b[:, 4 * g : 4 * g + 4], in_=rstd_b[:, 4 * g : 4 * g + 4],
                    func=AF.Exp, bias=0.0, scale=-0.5,
                )
                for u in range(2 * g, 2 * g + 2):
                    z2 = work.tile([128, C], F32, tag="z2")
                    nc.vector.tensor_scalar(
                        out=z2[:],
                        in0=r2[u][:],
                        scalar1=mv_b[:, 2 * u : 2 * u + 1],
                        scalar2=rstd_b[:, 2 * u + 1 : 2 * u + 2],
                        op0=ALU.subtract,
                        op1=ALU.mult,
                    )
                    o1 = work.tile([128, C], F32, tag="o1")
                    nc.gpsimd.tensor_mul(o1[:], z2[:], g2b[:])
                    o = work.tile([128, C], BF16, tag="o")
                    nc.gpsimd.tensor_add(o[:], o1[:], b2lb[:])
                    nc.sync.dma_start(out=d["out"][128 * u : 128 * (u + 1), :], in_=o[:])

        proj_ln1(7)
        mlp1_piece(0)
        for t in range(4, 8):
            z1_fin(t)
        for t in range(4):
            mlp2_tile(t)
        mlp1_piece(1)
        for t in range(4, 8):
            mlp2_tile(t)
        ps_m2_cm.__exit__(None, None, None)
        ps_h_cm.__exit__(None, None, None)


_NC_CACHE = None
_LAST_RESULT = None


def _get_nc():
    global _NC_CACHE
    if _NC_CACHE is None:
        _NC_CACHE = _build_nc()
    return _NC_CACHE


def _to_bf16(a):
    return np.ascontiguousarray(np.asarray(a, dtype=np.float32)).astype(
        ml_dtypes.bfloat16
    )


def _host_inputs(core, x, mask, qkv_w, qkv_b, proj_w, proj_b, ln1_g, ln1_b, w1,
                 b1, w2, b2, ln2_g, ln2_b):
    b = core // 2
    half = core % 2
    row0 = 16 * half - PAD_ROWS  # slab start image row (may be negative)
    S0 = row0 * W_IMG  # slab start token
    Q0g = 1024 * half  # first query token (global)

    xb = np.asarray(x[b], dtype=np.float32)  # [N, C]
    slab = np.zeros((T, C), np.float32)
    g_lo, g_hi = max(0, S0), min(N, S0 + T)
    slab[g_lo - S0 : g_hi - S0] = xb[g_lo:g_hi]

    wqk = np.concatenate([qkv_w[:C] * SCALE, qkv_w[C : 2 * C]], axis=0)  # [512,C]
    qkb = np.concatenate([qkv_b[:C] * SCALE, qkv_b[C : 2 * C]])[:, None]
    wv = qkv_w[2 * C :]  # [256, 256]
    vb = qkv_b[2 * C :]
    assert np.abs(vb).max() == 0.0, "nonzero v bias not folded"
    wv_pad = np.zeros((C, 264), np.float32)
    for h in range(NH):
        wv_pad[:, 33 * h : 33 * h + 32] = wv[32 * h : 32 * h + 32].T

    w1f = w1 * ln1_g[None, :]  # fold ln1 gamma
    b1f = (b1 + w1 @ ln1_b)[:, None]  # fold ln1 beta (mlp path)
    bvec2 = b2 + ln1_b  # resid2 constant (residual path)

    xres = xb[Q0g : Q0g + NQ] + proj_b[None, :]

    # {0,1} masks: 5 tiles [128, 512] (qt0, qt1, interior, qt6, qt7);
    # v[p, 128c+q] = valid(key (c,p), query q)
    v5 = np.zeros((5, 128, KW), np.float32)

    def _vt_of(i):
        qg = Q0g + 128 * i
        valid = np.zeros((128, KW), np.float32)  # [q, k-in-window]
        for r in range(8):
            gr = row0 + 2 * i + r  # global image row of window row r
            if 0 <= gr < H_IMG:
                valid[:, 64 * r : 64 * (r + 1)] = (
                    mask[qg : qg + 128, 64 * gr : 64 * (gr + 1)] == 0
                )
        # coverage check: every allowed key lies inside the window
        full = mask[qg : qg + 128] == 0
        assert int(full.sum()) == int(valid.sum()), (core, i, "window coverage")
        return valid.T.reshape(4, 128, 128).transpose(1, 0, 2).reshape(128, KW)

    for sl, i in enumerate((0, 1, 2, 6, 7)):
        v5[sl] = _vt_of(i)
    for i in (3, 4, 5):  # interior coherence
        assert (v5[2] == _vt_of(i)).all(), (core, i, "interior mask mismatch")

    rows = np.concatenate([bvec2, ln2_g, ln2_b])[None, :]

    return {
        "xT": _to_bf16(slab.T),
        "xres": _to_bf16(xres),
        "wqk": _to_bf16(wqk.T),
        "qkb": np.ascontiguousarray(qkb, dtype=np.float32),
        "wv": _to_bf16(wv_pad),
        "wp": _to_bf16(proj_w.T),
        "dg1": _to_bf16(np.diag(ln1_g)),
        "w1": _to_bf16(w1f.T),
        "b1": np.ascontiguousarray(b1f, dtype=np.float32),
        "w2": _to_bf16(w2.T),
        "rows": np.ascontiguousarray(rows, dtype=np.float32),
        "m01": _to_bf16(v5),
    }


def kernel(**inputs):
    args = {k: np.asarray(v) for k, v in inputs.items()}
    in_maps = [
        _host_inputs(
            core,
            args["x"],
            np.asarray(args["mask"], dtype=np.float32),
            args["qkv_w"],
            args["qkv_b"],
            args["proj_w"],
            args["proj_b"],
            args["ln1_g"],
            args["ln1_b"],
            args["w1"],
            args["b1"],
            args["w2"],
            args["b2"],
            args["ln2_g"],
            args["ln2_b"],
        )
        for core in range(8)
    ]
    nc = _get_nc()
    res = run_bass_kernel_spmd(nc, in_maps, core_ids=list(range(8)))
    global _LAST_RESULT
    _LAST_RESULT = res
    out = np.zeros((B, N, C), np.float32)
    for core in range(8):
        b, half = core // 2, core % 2
        out[b, 1024 * half : 1024 * (half + 1)] = np.asarray(
            res.results[core]["out"], dtype=np.float32
        )
    return out


# revision 43
# speedup vs baseline: 1.0433x; 1.0369x over previous
"""Trainium2 Bass kernel for nn_MixingBlock (local-window attention + MLP).

Sharding: 8 cores = (batch 0..3) x (token half 0..1); each core computes
1024 output tokens of one batch element. Attention is local (7x11 window
on the 32x64 token grid), so each core works on a zero-padded 22-image-row
slab (T=1408 tokens) of x and needs no collectives: slab rows for half h
are global rows [16h-3, 16h+19), zero-padded outside [0,32). Queries sit
at slab tokens [192, 1216) and the key window of query tile i is slab
tokens [128i, 128i+512) for BOTH halves, so one SPMD program serves all
cores; window masking uses 5 host-built [128,512] tiles (qt0,qt1,
interior,qt6,qt7 -- edge tiles equal interior on the halves that don't
need them, keeping the program uniform). Per query tile, half the score
PSUMs get an additive -30000 mask folded in by identity matmuls (PE) and
half get a multiplicative {0,1} mask after the exp (split Vector/GpSimd)
to balance engine load.

Device layouts: Q^T/K^T channels-on-partitions ([512f, T], Q pre-scaled),
V token-major with a ones column per head ([T, 33*8]); scores are built
transposed (S^T = K Q^T) so softmax weights feed the AV matmul as
stationary operands without transposes; the ones column yields softmax
denominators inside the same accumulation (all 8 heads of a query tile
accumulate into one [128,264] PSUM; one strided reciprocal + one
broadcast multiply normalize all heads). MLP/LN run token-major
(bn_stats over channels, rstd batched as exp(-0.5*ln(var+eps)) over 4
tiles at a time) with one PE transpose of z1; the attention residual
rides the PSUM->SBUF staging op, and the LN1-gamma residual term rides
the mlp2 matmul via a diagonal gamma matrix. Output is shipped bf16 and
upcast on host.
"""

import contextlib
import sys
import types

import ml_dtypes
import numpy as np

import concourse.bass as bass
import concourse.mybir as mybir
import concourse.tile as tile

# ---------------------------------------------------------------------------
# axon NTFF profile hook (lets run_bass_kernel_spmd(trace=True) work here)
# ---------------------------------------------------------------------------
if "antenv.axon_hooks" not in sys.modules:
    try:
        import antenv  # noqa: F401

        _hookmod = types.ModuleType("antenv.axon_hooks")
        _hookmod._hook = None
        _hookmod.set_axon_ntff_profile_hook = lambda h: setattr(_hookmod, "_hook", h)
        _hookmod.get_axon_ntff_profile_hook = lambda: _hookmod._hook
        sys.modules["antenv.axon_hooks"] = _hookmod
        try:
            from trn_agent_boot.trn_boot import _ntff_profile_via_ctypes

            _hookmod.set_axon_ntff_profile_hook(
                _ntff_profile_via_ctypes("/opt/axon/libaxon_pjrt.so")
            )
        except Exception:
            pass
    except Exception:
        pass

from concourse.bass_utils import run_bass_kernel_spmd  # noqa: E402
from concourse.masks import make_identity  # noqa: E402
from concourse.tile_rust import add_dep_helper  # noqa: E402

F32 = mybir.dt.float32
BF16 = mybir.dt.bfloat16
AF = mybir.ActivationFunctionType
ALU = mybir.AluOpType

# Problem constants
H_IMG, W_IMG = 32, 64
N = H_IMG * W_IMG  # 2048
C = 256
NH = 8
HD = 32
HIDDEN = 1024
SCALE = HD**-0.5
B = 4
LN_EPS = 1e-5

# Sharding constants
PAD_ROWS = 3
SLAB_ROWS = 16 + 2 * PAD_ROWS  # 22
T = SLAB_ROWS * W_IMG  # 1408
NQ = 1024
NQT = 8
KW = 512  # key window tokens per query tile
Q0 = PAD_ROWS * W_IMG  # 192: first query token within slab
XCH = (512, 512, 384)  # xT column chunks (token ranges)

MAX_WAITS = 1


def _split_excess_waits(nc):
    """walrus accepts only MAX_WAITS sem-waits per instruction; move excess
    onto same-engine nops inserted immediately before the instruction."""
    for f in nc.m.functions:
        for bb in f.blocks:
            i = 0
            while i < len(bb.instructions):
                ins = bb.instructions[i]
                si = ins.sync_info
                if si is not None and si.on_wait and len(si.on_wait) > MAX_WAITS:
                    waits = list(si.on_wait)
                    extra, keep = waits[:-MAX_WAITS], waits[-MAX_WAITS:]
                    ins.sync_info = mybir.SyncInfo(
                        on_wait=keep, on_update=list(si.on_update)
                    )
                    nops = []
                    for j in range(0, len(extra), MAX_WAITS):
                        nop = nc.engines[ins.engine].nop().ins
                        cur = nc.cur_bb.bb
                        assert cur.instructions[-1] is nop
                        nop.sync_info = mybir.SyncInfo(
                            on_wait=extra[j : j + MAX_WAITS], on_update=[]
                        )
                        cur.instructions.pop()
                        nops.append(nop)
                    bb.instructions[i:i] = nops
                    i += len(nops)
                i += 1


def _build_nc():
    nc = bass.Bass("TRN2", target_bir_lowering=False, num_devices=8)

    d = {}
    d["xT"] = nc.dram_tensor("xT", [C, T], BF16, kind="ExternalInput")
    d["xres"] = nc.dram_tensor("xres", [NQ, C], BF16, kind="ExternalInput")
    d["wqk"] = nc.dram_tensor("wqk", [C, 512], BF16, kind="ExternalInput")
    d["qkb"] = nc.dram_tensor("qkb", [512, 1], F32, kind="ExternalInput")
    d["wv"] = nc.dram_tensor("wv", [C, 264], BF16, kind="ExternalInput")
    d["wp"] = nc.dram_tensor("wp", [C, C], BF16, kind="ExternalInput")
    d["dg1"] = nc.dram_tensor("dg1", [C, C], BF16, kind="ExternalInput")
    d["w1"] = nc.dram_tensor("w1", [C, HIDDEN], BF16, kind="ExternalInput")
    d["b1"] = nc.dram_tensor("b1", [HIDDEN, 1], F32, kind="ExternalInput")
    d["w2"] = nc.dram_tensor("w2", [HIDDEN, C], BF16, kind="ExternalInput")
    d["rows"] = nc.dram_tensor("rows", [1, 3 * C], F32, kind="ExternalInput")
    d["m01"] = nc.dram_tensor("m01", [5, 128, KW], BF16, kind="ExternalInput")
    d["out"] = nc.dram_tensor("out", [NQ, C], BF16, kind="ExternalOutput")

    with tile.TileContext(nc) as tc:
        _emit(nc, tc, d)

    _split_excess_waits(nc)
    return nc


# mask tile index per query tile: qt0, qt1, interior x4, qt6, qt7
M01_OF_QT = [0, 1, 2, 2, 2, 2, 3, 4]


def _emit(nc, tc, d):
    ctx = contextlib.ExitStack()
    with ctx:
        const = ctx.enter_context(tc.tile_pool(name="const", bufs=1))
        big = ctx.enter_context(tc.tile_pool(name="big", bufs=1))
        ps = ctx.enter_context(tc.tile_pool(name="ps", bufs=1, space="PSUM"))
        ps_av = ctx.enter_context(tc.tile_pool(name="ps_av", bufs=1, space="PSUM"))
        ps_t = ps  # transposes/proj ride the 1-buf "mm" ring
        ps_p = ps
        P = {}  # phase-scoped psum pools: "s" (attention) / "h","m2" (mlp)
        work = ctx.enter_context(tc.tile_pool(name="work", bufs=5))
        small = ctx.enter_context(tc.tile_pool(name="small", bufs=6))

        late_dmas = []
        # ---------------- inputs to SBUF (issue order = first use) -------
        # xT row-tiles split into column chunks so phase 1 starts early.
        xT = [
            [
                const.tile([128, w], BF16, tag=f"xT{i}_{c}", name=f"xT{i}_{c}")
                for c, w in enumerate(XCH)
            ]
            for i in range(2)
        ]
        wqk = [const.tile([128, 512], BF16, tag=f"wqk{i}", name=f"wqk{i}") for i in range(2)]
        wv = [const.tile([128, 264], BF16, tag=f"wv{i}", name=f"wv{i}") for i in range(2)]
        wp = [const.tile([128, C], BF16, tag=f"wp{i}", name=f"wp{i}") for i in range(2)]
        dg1 = [const.tile([128, C], BF16, tag=f"dg1{i}", name=f"dg1{i}") for i in range(2)]
        w1 = [const.tile([128, HIDDEN], BF16, tag=f"w1{i}", name=f"w1s{i}") for i in range(2)]
        qkb = [const.tile([128, 1], F32, tag=f"qkb{i}", name=f"qkb{i}") for i in range(4)]
        b1 = [const.tile([128, 1], F32, tag=f"b1{i}", name=f"b1s{i}") for i in range(8)]
        w2 = [const.tile([128, C], BF16, tag=f"w2{i}", name=f"w2s{i}") for i in range(8)]
        xres = [const.tile([128, C], BF16, tag=f"xres{i}", name=f"xres{i}") for i in range(8)]
        m01 = [
            const.tile([128, KW], BF16, tag=f"m01_{i}", name=f"m01_{i}")
            for i in range(5)
        ]
        m30 = [
            const.tile([128, KW], BF16, tag=f"m30_{i}", name=f"m30_{i}")
            for i in range(5)
        ]
        # chunk 0 of xT first (gates the first matmul), wqk/qkb in parallel
        # on the scalar queue.
        off = 0
        for c, w in enumerate(XCH):
            for i in range(2):
                nc.sync.dma_start(out=xT[i][c][:], in_=d["xT"][128 * i : 128 * (i + 1), off : off + w])
            off += w
        for i in range(2):
            nc.scalar.dma_start(out=wqk[i][:], in_=d["wqk"][128 * i : 128 * (i + 1), :])
        for i in range(4):
            nc.scalar.dma_start(out=qkb[i][:], in_=d["qkb"][128 * i : 128 * (i + 1), :])
        for i in range(2):
            nc.gpsimd.dma_start(out=wv[i][:], in_=d["wv"][128 * i : 128 * (i + 1), :])
        for i in range(5):
            late_dmas.append(nc.scalar.dma_start(out=m01[i][:], in_=d["m01"][i]).ins)
        for i in range(2):
            late_dmas.append(nc.scalar.dma_start(out=wp[i][:], in_=d["wp"][128 * i : 128 * (i + 1), :]).ins)
        for i in range(2):
            late_dmas.append(nc.scalar.dma_start(out=dg1[i][:], in_=d["dg1"][128 * i : 128 * (i + 1), :]).ins)
        for i in range(8):
            late_dmas.append(nc.sync.dma_start(
                out=xres[i][:], in_=d["xres"][128 * i : 128 * (i + 1), :]
            ).ins)
        for i in range(2):
            late_dmas.append(nc.sync.dma_start(out=w1[i][:], in_=d["w1"][128 * i : 128 * (i + 1), :]).ins)
        for i in range(8):
            late_dmas.append(nc.sync.dma_start(out=b1[i][:], in_=d["b1"][128 * i : 128 * (i + 1), :]).ins)
        for i in range(8):
            late_dmas.append(nc.sync.dma_start(out=w2[i][:], in_=d["w2"][128 * i : 128 * (i + 1), :]).ins)
        g1b = const.tile([128, C], F32)
        g2b = const.tile([128, C], F32)
        b2lb = const.tile([128, C], F32)
        for t_, lo in ((g1b, 0), (g2b, C), (b2lb, 2 * C)):
            late_dmas.append(
                nc.sync.dma_start(
                    out=t_[:], in_=d["rows"][0:1, lo : lo + C].to_broadcast((128, C))
                ).ins
            )

        eps_t = const.tile([128, 1], F32)
        nc.vector.memset(eps_t[:], LN_EPS)
        ident = const.tile([128, 128], BF16)
        make_identity(nc, ident)
        # additive -30000/0 masks derived on-device from the {0,1} masks
        for i in range(5):
            nc.vector.tensor_scalar(
                out=m30[i][:], in0=m01[i][:], scalar1=-1.0, scalar2=30000.0,
                op0=ALU.add, op1=ALU.mult,
            )

        # ---------------- phase 1: Q^T (scaled) and K^T ----------------
        # K rows (m=2,3) per xT chunk as the chunks arrive; Q rows after.
        qkT = [big.tile([128, T], BF16, tag=f"qkT{m}", name=f"qkT{m}") for m in range(4)]
        first_mm = [None]

        def qk_piece(m, lo, hi, chunk):
            p = P["s"].tile([128, 512], F32, tag="s_ps", name="p_qk")
            w = hi - lo
            coff = sum(XCH[:chunk])
            for cc in range(2):
                mm0 = nc.tensor.matmul(
                    p[:, :w],
                    wqk[cc][:, 128 * m : 128 * (m + 1)],
                    xT[cc][chunk][:, lo - coff : hi - coff],
                    start=(cc == 0),
                    stop=(cc == 1),
                )
                if first_mm[0] is None:
                    first_mm[0] = mm0
                    for dma in late_dmas:
                        add_dep_helper(dma, mm0.ins, sync=True,
                                       reason="defer bulk input DMA")
            nc.vector.tensor_scalar_add(
                out=qkT[m][:, lo:hi], in0=p[:, :w], scalar1=qkb[m][:]
            )

        # ---------------- phase 2: V (token-major, ones columns) ----------
        vt = [big.tile([128, 264], BF16, tag=f"vt{i}", name=f"vt{i}") for i in range(T // 128)]

        def v_tile(i):
            p = P["s"].tile([128, 264], F32, tag="s_ps", name="p_v")
            chunk, coff = i // 4, 128 * (i % 4)
            for cc in range(2):
                nc.tensor.matmul(
                    p[:, :264],
                    xT[cc][chunk][:, coff : coff + 128],
                    wv[cc][:],
                    start=(cc == 0),
                    stop=(cc == 1),
                )
            nc.vector.tensor_copy(vt[i][:], p[:, :264])
            nc.gpsimd.memset(vt[i][:, 32::33], 1.0)

        # ---------------- phase 3: attention ----------------
        attnT = [
            [
                big.tile([128, 128], BF16, tag=f"attnT{j}_{q}", name=f"attnT{j}_{q}")
                for q in range(NQT)
            ]
            for j in range(2)
        ]
        # phase 4 bodies, interleaved into the qt loop (proj of tile t runs
        # while attention works on tile t+1) to keep the PE warm.
        z1 = [big.tile([128, C], BF16, tag=f"z1{i}", name=f"z1_{i}") for i in range(8)]
        z1u = [big.tile([128, C], F32, tag=f"z1u{i}", name=f"z1u_{i}") for i in range(8)]
        mv_a = const.tile([128, 16], F32)
        rstd_a = const.tile([128, 16], F32)

        def proj_ln1(t):
            p_p = ps_p.tile([128, C], F32, tag="mm", name="p_p")
            for cc in range(2):
                nc.tensor.matmul(
                    p_p[:, :C],
                    attnT[cc][t][:],
                    wp[cc][:],
                    start=(cc == 0),
                    stop=(cc == 1),
                )
            nc.vector.scalar_tensor_tensor(
                out=z1u[t][:], in0=p_p[:, :C], scalar=1.0, in1=xres[t][:],
                op0=ALU.mult, op1=ALU.add,
            )
            stats = small.tile([128, 6], F32, tag="stats")
            nc.vector.bn_stats(out=stats[:], in_=z1u[t][:])
            nc.vector.bn_aggr(out=mv_a[:, 2 * t : 2 * t + 2], in_=stats[:])
            if t % 4 == 3:
                g = t // 4
                nc.scalar.activation(
                    out=rstd_a[:, 8 * g : 8 * g + 8], in_=mv_a[:, 8 * g : 8 * g + 8],
                    func=AF.Ln, bias=eps_t[:], scale=1.0,
                )
                nc.scalar.activation(
                    out=rstd_a[:, 8 * g : 8 * g + 8], in_=rstd_a[:, 8 * g : 8 * g + 8],
                    func=AF.Exp, bias=0.0, scale=-0.5,
                )

        z1T = [
            [
                big.tile([128, 512], BF16, tag=f"z1T{j}_{p}", name=f"z1T{j}_{p}")
                for p in range(2)
            ]
            for j in range(2)
        ]

        def z1_fin(t):
            nc.vector.tensor_scalar(
                out=z1[t][:],
                in0=z1u[t][:],
                scalar1=mv_a[:, 2 * t : 2 * t + 1],
                scalar2=rstd_a[:, 2 * t + 1 : 2 * t + 2],
                op0=ALU.subtract,
                op1=ALU.mult,
            )
            for j in range(2):
                p_t = ps_t.tile([128, 128], BF16, tag="mm", name="p_t")
                nc.tensor.transpose(
                    p_t[:, :128], z1[t][:, 128 * j : 128 * (j + 1)], ident[:]
                )
                nc.vector.tensor_copy(
                    z1T[j][t // 4][:, 128 * (t % 4) : 128 * (t % 4 + 1)],
                    p_t[:, :128],
                )

        hT = [
            big.tile([128, 1024], BF16, tag=f"hT{i}", name=f"hT{i}")
            for i in range(8)
        ]

        def mlp1_piece(piece):
            for hc in range(8):
                p_h = P["h"].tile([128, 512], F32, tag="h", name="p_h")
                for cc in range(2):
                    nc.tensor.matmul(
                        p_h[:, :512],
                        w1[cc][:, 128 * hc : 128 * (hc + 1)],
                        z1T[cc][piece][:],
                        start=(cc == 0),
                        stop=(cc == 1),
                    )
                nc.scalar.activation(
                    out=hT[hc][:, 512 * piece : 512 * piece + 512],
                    in_=p_h[:, :512],
                    func=AF.Gelu,
                    bias=b1[hc][:],
                    scale=1.0,
                )

        def attention(qt):
            kw0 = 128 * qt  # key window start token in slab
            attn_q = work.tile([128, NH, 32], BF16, tag="attn_q", name="attn_q")
            p_av8 = ps_av.tile([128, 264], F32, tag="av", name="p_av8")
            for hq in range(2):  # two groups of 4 heads
                heads = [4 * hq + j for j in range(4)]
                p_sA = P["s"].tile([128, 2, KW], F32, tag="s_ps", name="p_sA")
                p_sB = P["s"].tile([128, 2, KW], F32, tag="s_ps", name="p_sB")
                p_of = {heads[0]: (p_sA, 0, False), heads[1]: (p_sA, 1, False),
                        heads[2]: (p_sB, 0, True), heads[3]: (p_sB, 1, True)}
                for c in range(4):
                    for h in (heads[0], heads[2], heads[1], heads[3]):
                        pt_, hi, is_b = p_of[h]
                        ktile, koff = 2 + h // 4, (32 * h) % 128
                        qtile, qoff = h // 4, (32 * h) % 128
                        nc.tensor.matmul(
                            pt_[:, hi, 128 * c : 128 * (c + 1)],
                            qkT[ktile][
                                koff : koff + 32,
                                kw0 + 128 * c : kw0 + 128 * (c + 1),
                            ],
                            qkT[qtile][
                                qoff : qoff + 32,
                                Q0 + 128 * qt : Q0 + 128 * (qt + 1),
                            ],
                            start=(c == 0),
                            stop=(c == 3) and is_b,
                            tile_position=(koff, 0),
                        )
                # additive mask folded into piece A via identity matmuls
                for hi in range(2):
                    nc.tensor.matmul(
                        p_sA[:, hi, :], ident[:], m30[M01_OF_QT[qt]][:],
                        start=False, stop=True,
                    )
                for pi, p_s in enumerate((p_sA, p_sB)):
                    pT = work.tile([128, 2, KW], BF16, tag="pT", name="pT")
                    nc.scalar.activation(
                        out=pT[:], in_=p_s[:], func=AF.Exp, bias=0.0, scale=1.0
                    )
                    if pi == 1:  # piece B: multiplicative {0,1} mask
                        meng = nc.vector if hq == 0 else nc.gpsimd
                        meng.tensor_mul(
                            pT[:],
                            pT[:],
                            m01[M01_OF_QT[qt]][:]
                            .rearrange("p (o k) -> p o k", o=1)
                            .to_broadcast((128, 2, KW)),
                        )
                    for hi in range(2):
                        h = heads[2 * pi + hi]
                        for c in range(4):
                            nc.tensor.matmul(
                                p_av8[:, 33 * h : 33 * h + 33],
                                pT[:, hi, 128 * c : 128 * (c + 1)],
                                vt[qt + c][:, 33 * h : 33 * h + 33],
                                start=(c == 0),
                                stop=(c == 3),
                            )
            rec8 = small.tile([128, 8], F32, tag="rec")
            nc.vector.reciprocal(rec8[:], p_av8[:, 32::33])
            nc.vector.tensor_mul(
                attn_q[:],
                p_av8[:].rearrange("p (h x) -> p h x", h=8)[:, :, 0:32],
                rec8[:].rearrange("p (h o) -> p h o", o=1).to_broadcast((128, 8, 32)),
            )
            for j in range(2):
                p_t2 = ps_t.tile([128, 128], BF16, tag="mm", name="p_t2")
                nc.tensor.transpose(
                    p_t2[:, :128],
                    attn_q[:, 4 * j : 4 * (j + 1), :].rearrange("p a b -> p (a b)"),
                    ident[:],
                )
                nc.vector.tensor_copy(attnT[j][qt][:], p_t2[:, :128])

        # ------------- emission schedule -------------
        ps_s_cm = tc.tile_pool(name="ps_s", bufs=3, space="PSUM")
        P["s"] = ps_s_cm.__enter__()
        qk_piece(2, 0, 512, 0)
        qk_piece(3, 0, 512, 0)
        qk_piece(2, 512, 1024, 1)
        qk_piece(3, 512, 1024, 1)
        qk_piece(0, Q0, 512, 0)
        qk_piece(1, Q0, 512, 0)
        qk_piece(0, 512, 1024, 1)
        qk_piece(1, 512, 1024, 1)
        qk_piece(2, 1024, T, 2)
        qk_piece(3, 1024, T, 2)
        qk_piece(0, 1024, Q0 + NQ, 2)
        qk_piece(1, 1024, Q0 + NQ, 2)
        for i in range(11):
            v_tile(i)
        attention(0)
        attention(1)
        proj_ln1(0)
        attention(2)
        proj_ln1(1)
        attention(3)
        proj_ln1(2)
        attention(4)
        proj_ln1(3)
        attention(5)
        proj_ln1(4)
        z1_fin(0)
        z1_fin(1)
        attention(6)
        proj_ln1(5)
        z1_fin(2)
        z1_fin(3)
        attention(7)
        proj_ln1(6)
        ps_s_cm.__exit__(None, None, None)
        ps_h_cm = tc.tile_pool(name="ps_h", bufs=3, space="PSUM")
        P["h"] = ps_h_cm.__enter__()
        ps_m2_cm = tc.tile_pool(name="ps_m2", bufs=3, space="PSUM")
        P["m2"] = ps_m2_cm.__enter__()
        # ---------------- phase 6: mlp2 + resid2 + LN2 + out -------------
        # p_m accumulates mlp2 AND the gamma1*z1 residual (diag matmul);
        # bv2 rides the PSUM->SBUF staging op.
        mv_b = const.tile([128, 16], F32)
        rstd_b = const.tile([128, 16], F32)
        r2 = [big.tile([128, C], F32, tag=f"r2_{t}", name=f"r2_{t}") for t in range(8)]

        def mlp2_tile(t):
            p_m = P["m2"].tile([128, C], F32, tag="m2", name="p_m")
            tok0 = 512 * (t // 4) + 128 * (t % 4)
            for hc in range(8):
                nc.tensor.matmul(
                    p_m[:, :C],
                    hT[hc][:, tok0 : tok0 + 128],
                    w2[hc][:],
                    start=(hc == 0),
                    stop=False,
                )
            for cc in range(2):
                nc.tensor.matmul(
                    p_m[:, :C],
                    z1T[cc][t // 4][:, 128 * (t % 4) : 128 * (t % 4 + 1)],
                    dg1[cc][:],
                    start=False,
                    stop=(cc == 1),
                )
            nc.vector.scalar_tensor_tensor(
                out=r2[t][:], in0=p_m[:, :C], scalar=1.0, in1=g1b[:],
                op0=ALU.mult, op1=ALU.add,
            )
            stats = small.tile([128, 6], F32, tag="stats2")
            nc.vector.bn_stats(out=stats[:], in_=r2[t][:])
            nc.vector.bn_aggr(out=mv_b[:, 2 * t : 2 * t + 2], in_=stats[:])
            if t % 2 == 1:
                g = t // 2
                nc.scalar.activation(
                    out=rstd_b[:, 4 * g : 4 * g + 4], in_=mv_b[:, 4 * g : 4 * g + 4],
                    func=AF.Ln, bias=eps_t[:], scale=1.0,
                )
                nc.scalar.activation(
                    out=rstd_b[:, 4 * g : 4 * g + 4], in_=rstd_b[:, 4 * g : 4 * g + 4],
                    func=AF.Exp, bias=0.0, scale=-0.5,
                )
                for u in range(2 * g, 2 * g + 2):
                    z2 = work.tile([128, C], F32, tag="z2")
                    nc.vector.tensor_scalar(
                        out=z2[:],
                        in0=r2[u][:],
                        scalar1=mv_b[:, 2 * u : 2 * u + 1],
                        scalar2=rstd_b[:, 2 * u + 1 : 2 * u + 2],
                        op0=ALU.subtract,
                        op1=ALU.mult,
                    )
                    o1 = work.tile([128, C], F32, tag="o1")
                    nc.gpsimd.tensor_mul(o1[:], z2[:], g2b[:])
                    o = work.tile([128, C], BF16, tag="o")
                    nc.gpsimd.tensor_add(o[:], o1[:], b2lb[:])
                    nc.sync.dma_start(out=d["out"][128 * u : 128 * (u + 1), :], in_=o[:])

        proj_ln1(7)
        mlp1_piece(0)
        for t in range(4, 8):
            z1_fin(t)
        for t in range(4):
            mlp2_tile(t)
        mlp1_piece(1)
        for t in range(4, 8):
            mlp2_tile(t)
        ps_m2_cm.__exit__(None, None, None)
        ps_h_cm.__exit__(None, None, None)


_NC_CACHE = None
_LAST_RESULT = None


def _get_nc():
    global _NC_CACHE
    if _NC_CACHE is None:
        _NC_CACHE = _build_nc()
    return _NC_CACHE


def _to_bf16(a):
    return np.ascontiguousarray(np.asarray(a, dtype=np.float32)).astype(
        ml_dtypes.bfloat16
    )


def _host_inputs(core, x, mask, qkv_w, qkv_b, proj_w, proj_b, ln1_g, ln1_b, w1,
                 b1, w2, b2, ln2_g, ln2_b):
    b = core // 2
    half = core % 2
    row0 = 16 * half - PAD_ROWS  # slab start image row (may be negative)
    S0 = row0 * W_IMG  # slab start token
    Q0g = 1024 * half  # first query token (global)

    xb = np.asarray(x[b], dtype=np.float32)  # [N, C]
    slab = np.zeros((T, C), np.float32)
    g_lo, g_hi = max(0, S0), min(N, S0 + T)
    slab[g_lo - S0 : g_hi - S0] = xb[g_lo:g_hi]

    wqk = np.concatenate([qkv_w[:C] * SCALE, qkv_w[C : 2 * C]], axis=0)  # [512,C]
    qkb = np.concatenate([qkv_b[:C] * SCALE, qkv_b[C : 2 * C]])[:, None]
    wv = qkv_w[2 * C :]  # [256, 256]
    vb = qkv_b[2 * C :]
    assert np.abs(vb).max() == 0.0, "nonzero v bias not folded"
    wv_pad = np.zeros((C, 264), np.float32)
    for h in range(NH):
        wv_pad[:, 33 * h : 33 * h + 32] = wv[32 * h : 32 * h + 32].T

    w1f = w1 * ln1_g[None, :]  # fold ln1 gamma
    b1f = (b1 + w1 @ ln1_b)[:, None]  # fold ln1 beta (mlp path)
    bvec2 = b2 + ln1_b  # resid2 constant (residual path)

    xres = xb[Q0g : Q0g + NQ] + proj_b[None, :]

    # {0,1} masks: 5 tiles [128, 512] (qt0, qt1, interior, qt6, qt7);
    # v[p, 128c+q] = valid(key (c,p), query q)
    v5 = np.zeros((5, 128, KW), np.float32)

    def _vt_of(i):
        qg = Q0g + 128 * i
        valid = np.zeros((128, KW), np.float32)  # [q, k-in-window]
        for r in range(8):
            gr = row0 + 2 * i + r  # global image row of window row r
            if 0 <= gr < H_IMG:
                valid[:, 64 * r : 64 * (r + 1)] = (
                    mask[qg : qg + 128, 64 * gr : 64 * (gr + 1)] == 0
                )
        # coverage check: every allowed key lies inside the window
        full = mask[qg : qg + 128] == 0
        assert int(full.sum()) == int(valid.sum()), (core, i, "window coverage")
        return valid.T.reshape(4, 128, 128).transpose(1, 0, 2).reshape(128, KW)

    for sl, i in enumerate((0, 1, 2, 6, 7)):
        v5[sl] = _vt_of(i)
    for i in (3, 4, 5):  # interior coherence
        assert (v5[2] == _vt_of(i)).all(), (core, i, "interior mask mismatch")

    rows = np.concatenate([bvec2, ln2_g, ln2_b])[None, :]

    return {
        "xT": _to_bf16(slab.T),
        "xres": _to_bf16(xres),
        "wqk": _to_bf16(wqk.T),
        "qkb": np.ascontiguousarray(qkb, dtype=np.float32),
        "wv": _to_bf16(wv_pad),
        "wp": _to_bf16(proj_w.T),
        "dg1": _to_bf16(np.diag(ln1_g)),
        "w1": _to_bf16(w1f.T),
        "b1": np.ascontiguousarray(b1f, dtype=np.float32),
        "w2": _to_bf16(w2.T),
        "rows": np.ascontiguousarray(rows, dtype=np.float32),
        "m01": _to_bf16(v5),
    }


def kernel(**inputs):
    args = {k: np.asarray(v) for k, v in inputs.items()}
    in_maps = [
        _host_inputs(
            core,
            args["x"],
            np.asarray(args["mask"], dtype=np.float32),
            args["qkv_w"],
            args["qkv_b"],
            args["proj_w"],
            args["proj_b"],
            args["ln1_g"],
            args["ln1_b"],
            args["w1"],
            args["b1"],
            args["w2"],
            args["b2"],
            args["ln2_g"],
            args["ln2_b"],
        )
        for core in range(8)
    ]
    nc = _get_nc()
    res = run_bass_kernel_spmd(nc, in_maps, core_ids=list(range(8)))
    global _LAST_RESULT
    _LAST_RESULT = res
    out = np.zeros((B, N, C), np.float32)
    for core in range(8):
        b, half = core // 2, core % 2
        out[b, 1024 * half : 1024 * (half + 1)] = np.asarray(
            res.results[core]["out"], dtype=np.float32
        )
    return out
